# revision 48
# baseline (speedup 1.0000x reference)
"""Trainium2 Bass kernel for the 3-layer KAN GRN block.

Sharding: all three KAN layers are sharded over their *output* dim across the
8 cores (fc1: 32 cols/core of DH=256; fc2/sk: 16 cols/core of DOUT=128).
fc2 consumes the full fc1 output via an in-kernel AllGather; a second
AllGather assembles the fc2 + skip outputs, after which the (cheap) GLU gate
and BatchNorm run replicated on every core.

Layout: the input dim of each KAN layer is split into 4 quarters, one per
32-row strip of the PE array (explicit tile_position row tiling).  Stage-1
(the K=8 feature contraction) runs one matmul per (i, k-chunk) with the
weights stationary on strip (i // di4); the four strips execute concurrently
on hardware and, more importantly, every weight/feats DMA now spreads over
4x more SBUF partitions.  silu runs on ACT straight out of PSUM in
[128,1024] tiles; stage-2 (the contraction over hidden H and input dim)
accumulates into a single PSUM tile via matmuls against a block-diagonal W2
that is shipped compact from HBM ([128, din*CH] true bytes) and expanded
on-chip into a persistent SBUF buffer with one small diagonal-fill DMA per
output column (the zero background is written once by memset).
"""

import sys

import numpy as np

for _p in ("/opt/trn_rl_repo",):
    if _p not in sys.path:
        sys.path.insert(0, _p)

import concourse.bass as bass  # noqa: E402
import concourse.tile as tile  # noqa: E402
from concourse import bacc, bass_utils, mybir  # noqa: E402

F32 = mybir.dt.float32
BF16 = mybir.dt.bfloat16
AF = mybir.ActivationFunctionType
ALU = mybir.AluOpType
NP_BF16 = mybir.dt.np(BF16)

ACT_FN = AF.Silu  # patched to AF.Sigmoid for CoreSim structural tests

NCORES = 8
B = 128
DIN = 128
DH = 256
DOUT = 128
H = 32
NSIN = 3
BN_EPS = 1e-5
TWO_PI = float(2.0 * np.pi)
MAGIC = float(1.5 * 2.0**23)  # fp32 round-to-nearest-int trick
NQ = 4  # PE row strips / input-dim quarters
IB = 8  # i values per quarter per weight DMA block

# layer geometry: (din, JL, HC, CH)
GEO_FC1 = (DIN, 32, 4, 8)
GEO_FC2 = (DH, 16, 8, 4)
GEO_SK = (DIN, 16, 8, 4)


def _feats_host(xt):
    """xt: [din, B] f32 -> [32, (din//4)*B] bf16: row (8g+f) = feature f of
    input-quarter g (f7 = const 1 for the B1 fold)."""
    din = xt.shape[0]
    comps = [xt]
    for s in range(NSIN):
        comps.append(np.sin((2.0**s) * xt))
    for s in range(NSIN):
        comps.append(np.cos((2.0**s) * xt))
    comps.append(np.ones_like(xt))
    arr = np.stack(comps, 0)  # [8, din, B]
    di4 = din // NQ
    arr = arr.reshape(8, NQ, di4 * B)  # [f, g, i*B]
    arr = np.transpose(arr, (1, 0, 2)).reshape(32, di4 * B)
    return np.ascontiguousarray(arr.astype(NP_BF16))


def _prep_w1(W1, B1, jsl, geo):
    """-> [32, nblk, IB*CH*128] bf16 lhsT blocks.

    Row (8g+f), block blk, free (isub, k, p=jl*HC+hl) =
      W1e[g*di4 + blk*IB + isub, jl, k*HC+hl, f]."""
    din, JL, HC, CH = geo
    di4 = din // NQ
    nblk = di4 // IB
    W1c = W1[:, jsl].astype(np.float32)  # [din, JL, H, 7]
    B1c = B1[:, jsl].astype(np.float32)  # [din, JL, H]
    W1e = np.concatenate([W1c, B1c[..., None]], axis=-1)  # [din, JL, H, 8]
    t = W1e.reshape(NQ, nblk, IB, JL, CH, HC, 8)  # [g,blk,isub,jl,k,hl,f]
    t = np.transpose(t, (0, 6, 1, 2, 4, 3, 5))  # [g,f,blk,isub,k,jl,hl]
    t = t.reshape(NQ * 8, nblk, IB * CH * 128)
    return np.ascontiguousarray(t.astype(NP_BF16))


def _prep_w2(W2, jsl, geo):
    """-> [128, din*CH] bf16 compact W2: row (jl*HC+hl), col (i*CH+k) =
    W2[i, jl, k*HC+hl]."""
    din, JL, HC, CH = geo
    W2c = W2[:, jsl, 0, :].astype(np.float32)  # [din, JL, H]
    t = W2c.reshape(din, JL, CH, HC)  # [i, jl, k, hl]
    t = np.transpose(t, (1, 3, 0, 2))  # [jl, hl, i, k]
    t = t.reshape(128, din * CH)
    return np.ascontiguousarray(t.astype(NP_BF16))


def _col(v):
    return np.ascontiguousarray(v.astype(np.float32).reshape(-1, 1))


def prep_in_maps(inputs):
    x = np.asarray(inputs["x"], np.float32)
    feats1 = _feats_host(np.ascontiguousarray(x.T))
    in_maps = []
    for c in range(NCORES):
        j1 = slice(32 * c, 32 * c + 32)
        j2 = slice(16 * c, 16 * c + 16)
        m = {
            "feats1": feats1,
            "w1fc1": _prep_w1(inputs["fc1_W1"], inputs["fc1_B1"], j1, GEO_FC1),
            "w2fc1": _prep_w2(inputs["fc1_W2"], j1, GEO_FC1),
            "w1fc2": _prep_w1(inputs["fc2_W1"], inputs["fc2_B1"], j2, GEO_FC2),
            "w2fc2": _prep_w2(inputs["fc2_W2"], j2, GEO_FC2),
            "w1sk": _prep_w1(inputs["sk_W1"], inputs["sk_B1"], j2, GEO_SK),
            "w2sk": _prep_w2(inputs["sk_W2"], j2, GEO_SK),
            "b2fc1": _col(inputs["fc1_B2"][:, j1, 0].sum(0)),
            "b2fc2": _col(inputs["fc2_B2"][:, j2, 0].sum(0)),
            "b2sk": _col(inputs["sk_B2"][:, j2, 0].sum(0)),
            "g1wT": np.ascontiguousarray(
                np.asarray(inputs["g1_w"], np.float32).T),
            "g2wT": np.ascontiguousarray(
                np.asarray(inputs["g2_w"], np.float32).T),
            "g1b": _col(inputs["g1_b"]),
            "g2b": _col(inputs["g2_b"]),
            "bnw": _col(inputs["bn_w"]),
            "bnb": _col(inputs["bn_b"]),
        }
        in_maps.append(m)
    return in_maps


def _emit_w2_expand(nc, e_t, w2_d, geo, split_memset=False):
    """memset e_t to 0 then fill the diagonal blocks from compact DRAM W2."""
    din, JL, HC, CH = geo
    dinCH = din * CH
    ef = e_t[:].bitcast(F32)  # f32 view halves the memset element count
    if split_memset:  # halve the zeroing latency again: DVE + Pool concur
        half = (JL // 2) * dinCH // 2
        nc.vector.memset(ef[:, 0:half], 0.0)
        nc.gpsimd.memset(ef[:, half:], 0.0)
    else:
        nc.gpsimd.memset(ef[:], 0.0)
    for jl in range(JL):
        eng = nc.sync if jl % 2 == 0 else nc.gpsimd
        eng.dma_start(
            e_t[jl * HC:(jl + 1) * HC, jl * dinCH:(jl + 1) * dinCH],
            w2_d.ap()[jl * HC:(jl + 1) * HC, :])


def _emit_w1_block(nc, w1p, w1_d, geo, blk):
    """DMA one W1 block (4 strip transfers) into a fresh pool tile."""
    din, JL, HC, CH = geo
    w1t = w1p.tile([128, IB * CH * 128], BF16, tag="w1")
    for g in range(NQ):
        nc.sync.dma_start(w1t[32 * g:32 * g + 8, :],
                          w1_d.ap()[8 * g:8 * g + 8, blk])
    return w1t


class _KanEmitter:
    """Streaming emitter for one KAN layer; blocks may be emitted in
    several groups (interleaved with other program phases).

    feats_t: [128, di4*B] bf16 (strip g rows 32g..32g+8 = features of
    quarter g).  w1_d: DRAM [32, nblk, IB*CH*128].  e_t: expanded W2
    [128, JL*din*CH] bf16 SBUF tile."""

    def __init__(self, tc, feats_t, w1_d, e_t, geo, name,
                 w1_pool=None):
        self.tc, self.nc = tc, tc.nc
        self.feats_t, self.w1_d, self.geo, self.name = feats_t, w1_d, geo, name
        din, JL, HC, CH = geo
        self.di4 = din // NQ
        self.nblk = self.di4 // IB
        self.kp = 2
        self.npacks = CH // self.kp
        self.ev = e_t[:].rearrange("p (j c) -> p c j", j=JL)
        self.n_acc = din * CH
        self.acc = 0
        if w1_pool is None:
            self.w1p_cm = tc.tile_pool(name=f"{name}_w1", bufs=2)
            self.w1p = self.w1p_cm.__enter__()
        else:
            self.w1p_cm, self.w1p = w1_pool
        self.o_cm = tc.tile_pool(name=f"{name}_o", bufs=1,
                                 space=bass.MemorySpace.PSUM)
        self.out2 = self.o_cm.__enter__().tile([JL, 128], F32)
        self.w1_pre = {}

    def prefetch(self, blocks):
        for blk in blocks:
            self.w1_pre[blk] = _emit_w1_block(self.nc, self.w1p, self.w1_d,
                                              self.geo, blk)

    def emit_blocks(self, blocks, a_bufs=6):
        nc, tc = self.nc, self.tc
        din, JL, HC, CH = self.geo
        kp, name = self.kp, self.name
        with tc.tile_pool(name=f"{name}_h", bufs=3,
                          space=bass.MemorySpace.PSUM) as hp_pool, \
             tc.tile_pool(name=f"{name}_a", bufs=a_bufs) as a_pool:
            ipp = 8 // CH  # isub's per pack (pack = 8 chunk slots, 1 strip)
            for blk in blocks:
                w1t = self.w1_pre.pop(blk, None)
                if w1t is None:
                    w1t = _emit_w1_block(nc, self.w1p, self.w1_d, self.geo,
                                         blk)
                for ig in range(IB // ipp):
                    for g in range(NQ):
                        # Whole pack on ONE PE row strip; consecutive packs
                        # rotate strips.  Mixing strips within the in-flight
                        # window while a matmul blocks on a semaphore crashes
                        # the PE (HW-verified), so strips only change at
                        # pack boundaries.
                        hp = hp_pool.tile([128, 1024], F32, tag="h")
                        for i2 in range(ipp):
                            isub = ig * ipp + i2
                            islot = blk * IB + isub
                            for k in range(CH):
                                s = i2 * CH + k
                                nc.tensor.matmul(
                                    hp[:, s * 128:(s + 1) * 128],
                                    w1t[32 * g:32 * g + 8,
                                        (isub * CH + k) * 128:
                                        (isub * CH + k + 1) * 128],
                                    self.feats_t[32 * g:32 * g + 8,
                                                 islot * 128:
                                                 (islot + 1) * 128],
                                    start=True, stop=True,
                                    tile_position=(32 * g, 0))
                        at = a_pool.tile([128, 1024], BF16, tag="a")
                        nc.scalar.activation(at[:], hp[:], ACT_FN)
                        for i2 in range(ipp):
                            isub = ig * ipp + i2
                            i = g * self.di4 + blk * IB + isub
                            for k in range(CH):
                                s = i2 * CH + k
                                nc.tensor.matmul(
                                    self.out2[:],
                                    self.ev[:, i * CH + k, :],
                                    at[:, s * 128:(s + 1) * 128],
                                    start=(self.acc == 0),
                                    stop=(self.acc == self.n_acc - 1))
                                self.acc += 1

    def finish(self, b2_t, out_sb, act_hook=None, close_w1=True):
        assert self.acc == self.n_acc
        if act_hook is None:
            self.nc.vector.tensor_scalar(out_sb, self.out2[:], b2_t, None,
                                         op0=ALU.add)
        else:
            act_hook(self.out2, b2_t, out_sb)
        self.o_cm.__exit__(None, None, None)
        if close_w1:
            self.w1p_cm.__exit__(None, None, None)


def _emit_kan_layer(tc, feats_t, w1_d, e_t, b2_t, out_sb, geo, name,
                    act_hook=None, prefetch=(), w1_pool=None, w1_pre=None,
                    a_bufs=6):
    em = _KanEmitter(tc, feats_t, w1_d, e_t, geo, name, w1_pool=w1_pool)
    if w1_pre:
        em.w1_pre = dict(w1_pre)
    if prefetch:
        em.prefetch(prefetch)
    em.emit_blocks(range(em.nblk), a_bufs=a_bufs)
    em.finish(b2_t, out_sb, act_hook)


def build_program():
    nc = bacc.Bacc("TRN2", target_bir_lowering=False, debug=False,
                   num_devices=NCORES)
    d = {}
    d["feats1"] = nc.dram_tensor("feats1", [32, (DIN // NQ) * B], BF16,
                                 kind="ExternalInput")
    d["w1fc1"] = nc.dram_tensor("w1fc1", [32, (DIN // NQ) // IB, IB * 8 * 128],
                                BF16, kind="ExternalInput")
    d["w2fc1"] = nc.dram_tensor("w2fc1", [128, DIN * 8], BF16,
                                kind="ExternalInput")
    d["w1fc2"] = nc.dram_tensor("w1fc2", [32, (DH // NQ) // IB, IB * 4 * 128],
                                BF16, kind="ExternalInput")
    d["w2fc2"] = nc.dram_tensor("w2fc2", [128, DH * 4], BF16,
                                kind="ExternalInput")
    d["w1sk"] = nc.dram_tensor("w1sk", [32, (DIN // NQ) // IB, IB * 4 * 128],
                               BF16, kind="ExternalInput")
    d["w2sk"] = nc.dram_tensor("w2sk", [128, DIN * 4], BF16,
                               kind="ExternalInput")
    for nm, rows in (("b2fc1", 32), ("b2fc2", 16), ("b2sk", 16), ("g1b", 128),
                     ("g2b", 128), ("bnw", 128), ("bnb", 128)):
        d[nm] = nc.dram_tensor(nm, [rows, 1], F32, kind="ExternalInput")
    d["g1wT"] = nc.dram_tensor("g1wT", [128, 128], F32, kind="ExternalInput")
    d["g2wT"] = nc.dram_tensor("g2wT", [128, 128], F32, kind="ExternalInput")
    out_d = nc.dram_tensor("outT", [DOUT, B], F32, kind="ExternalOutput")

    groups = [list(range(NCORES))]

    with tile.TileContext(nc) as tc:
        with tc.tile_pool(name="const", bufs=1) as cpool, \
             tc.tile_pool(name="feats", bufs=1) as fpool, \
             tc.tile_pool(name="big", bufs=1) as bigp, \
             tc.tile_pool(name="dram", bufs=2,
                          space=bass.MemorySpace.DRAM) as dpool, \
             tc.tile_pool(name="glue", bufs=2) as glue:
            # --- feats for fc1/sk (host-computed), 4 strip DMAs
            feats1_t = fpool.tile([128, (DIN // NQ) * B], BF16, tag="f1")
            for g in range(NQ):
                nc.sync.dma_start(feats1_t[32 * g:32 * g + 8, :],
                                  d["feats1"].ap()[8 * g:8 * g + 8, :])

            # --- sk emitter: first half runs before fc1 (covers the E1
            # expansion), second half after AG1 (covers the collective)
            ep_sk_cm = tc.tile_pool(name="w2e_sk", bufs=1)
            e_sk = ep_sk_cm.__enter__().tile([128, 16 * DIN * 4], BF16,
                                             tag="esk")
            sk_em = _KanEmitter(tc, feats1_t, d["w1sk"], e_sk, GEO_SK, "sk")
            sk_em.prefetch([0, 1])
            _emit_w2_expand(nc, e_sk, d["w2sk"], GEO_SK)

            # --- prefetch fc1's first two W1 blocks BEFORE the E1 fills so
            # stage-1 is never queued behind them on SP
            fc1_em = None  # created after prefetches to keep SP queue clean
            w1p_fc1_cm = tc.tile_pool(name="fc1_w1", bufs=2)
            w1p_fc1 = w1p_fc1_cm.__enter__()
            fc1_pre = {blk: _emit_w1_block(nc, w1p_fc1, d["w1fc1"], GEO_FC1,
                                           blk) for blk in range(2)}

            # --- expanded block-diagonal W2 for fc1
            ep_fc1_cm = tc.tile_pool(name="w2e_fc1", bufs=1)
            e_fc1 = ep_fc1_cm.__enter__().tile([128, 32 * DIN * 8], BF16,
                                               tag="efc1")
            _emit_w2_expand(nc, e_fc1, d["w2fc1"], GEO_FC1, split_memset=True)

            # --- constants (all consumed at layer tails — emit after the
            # start-critical DMA streams so they never delay them)
            bias = {}
            for nm, rows in (("b2fc1", 32), ("b2fc2", 16), ("b2sk", 16),
                             ("g1b", 128), ("g2b", 128), ("bnw", 128),
                             ("bnb", 128)):
                t = cpool.tile([rows, 1], F32, tag=nm)
                nc.sync.dma_start(t[:], d[nm].ap())
                bias[nm] = t
            g1w_t = cpool.tile([128, 128], F32, tag="g1w")
            nc.sync.dma_start(g1w_t[:], d["g1wT"].ap())
            g2w_t = cpool.tile([128, 128], F32, tag="g2w")
            nc.sync.dma_start(g2w_t[:], d["g2wT"].ap())


            # --- sk first half (covers the E1 build latency)
            sk_em.emit_blocks([0, 1], a_bufs=12)

            # --- fc1 (out cols 32c..32c+32), elu fused into output hook
            h1c = bigp.tile([32, 128], F32, tag="h1c")

            def elu_hook(out2, b2_t, out_sb):
                z = glue.tile([32, 128], F32, tag="eluz")
                nc.vector.tensor_scalar(z[:], out2[:], b2_t, None, op0=ALU.add)
                zn = glue.tile([32, 128], F32, tag="elun")
                nc.vector.tensor_scalar(zn[:], z[:], 0.0, None, op0=ALU.min)
                e = glue.tile([32, 128], F32, tag="elue")
                nc.scalar.activation(e[:], zn[:], AF.Exp)
                r = glue.tile([32, 128], F32, tag="elur")
                nc.vector.tensor_scalar(r[:], z[:], 0.0, None, op0=ALU.max)
                nc.vector.scalar_tensor_tensor(
                    out_sb, e[:], -1.0, r[:], op0=ALU.add, op1=ALU.add)

            fc1_em = _KanEmitter(tc, feats1_t, d["w1fc1"], e_fc1, GEO_FC1,
                                 "fc1", w1_pool=(w1p_fc1_cm, w1p_fc1))
            fc1_em.w1_pre = fc1_pre
            fc1_em.emit_blocks(range(fc1_em.nblk), a_bufs=8)
            fc1_em.finish(bias["b2fc1"][:], h1c[:], act_hook=elu_hook,
                          close_w1=False)
            ep_fc1_cm.__exit__(None, None, None)  # e_fc1 dead; reuse space
            w1p_fc1_cm.__exit__(None, None, None)

            # --- expand fc2's W2 (fills run under AG1 / sk second half),
            # and prefetch fc2's first W1 blocks in the same window
            ep_fc2_cm = tc.tile_pool(name="w2e_fc2", bufs=1)
            e_fc2 = ep_fc2_cm.__enter__().tile([128, 16 * DH * 4], BF16,
                                               tag="efc2")
            _emit_w2_expand(nc, e_fc2, d["w2fc2"], GEO_FC2)
            w1p_fc2_cm = tc.tile_pool(name="fc2_w1", bufs=3)
            w1p_fc2 = w1p_fc2_cm.__enter__()
            fc2_pre = {blk: _emit_w1_block(nc, w1p_fc2, d["w1fc2"], GEO_FC2,
                                           blk) for blk in range(2)}

            # --- AllGather #1: h1c [32,B] -> h1 [DH,B]
            ag1_in = dpool.tile([32, 128], F32, tag="ag1i")
            ag1_out = dpool.tile([DH, 128], F32, tag="ag1o")
            nc.sync.dma_start(ag1_in[:], h1c[:])
            nc.gpsimd.collective_compute(
                "AllGather", ALU.bypass, replica_groups=groups,
                ins=[ag1_in[:]], outs=[ag1_out[:]])

            # --- sk second half (covers the AllGather + trig chain)
            skc = bigp.tile([32, 128], F32, tag="catc")
            sk_tmp = bigp.tile([16, 128], F32, tag="sktmp")
            sk_em.emit_blocks([2, 3])
            sk_em.finish(bias["b2sk"][:], sk_tmp[:], close_w1=False)
            nc.sync.dma_start(skc[16:32, :], sk_tmp[:])

            # --- fc2 feature build (on-device trig with range reduction)
            # h1t: AG1 result as 2 tiles of [128(i), 128(b)] f32
            h1t = [bigp.tile([128, 128], F32, tag=f"h1t{hf}", name=f"h1t{hf}")
                   for hf in range(2)]
            for hf in range(2):
                nc.sync.dma_start(h1t[hf][:],
                                  ag1_out[128 * hf:128 * hf + 128, :])
            # feats2 strip g rows 32g+f = feature f of i in [64g, 64g+64)
            feats2_t = fpool.tile([128, (DH // NQ) * B], BF16, tag="f2")
            nc.gpsimd.memset(feats2_t[:], 1.0)  # const-1 row background
            # bounce DRAM tensor: row f = feature f, free (i in 256, b);
            # only 7 feature rows — the const-1 row stays from the memset
            f2d = dpool.tile([7, DH * B], BF16, tag="f2d")

            def store_feat_row(s, hf, src_bf16):
                # src [128(i), 128(b)] -> f2d[s, hf*16384 + i*128 + b]
                dst = f2d[s:s + 1, hf * 16384:(hf + 1) * 16384]
                vt = dst.rearrange("one (p b) -> (one p) b", p=128)
                nc.sync.dma_start(vt, src_bf16)

            for hf in range(2):
                c0 = glue.tile([128, 128], BF16, tag="c0")
                nc.vector.tensor_copy(c0[:], h1t[hf][:])
                store_feat_row(0, hf, c0[:])
                for s in range(NSIN):
                    sc = (2.0**s) / TWO_PI
                    y = glue.tile([128, 128], F32, tag="rry")
                    nc.vector.tensor_scalar(y[:], h1t[hf][:], sc, None,
                                            op0=ALU.mult)
                    kq = glue.tile([128, 128], F32, tag="rrk")
                    nc.vector.tensor_scalar(kq[:], y[:], MAGIC, -MAGIC,
                                            op0=ALU.add, op1=ALU.add)
                    fr = glue.tile([128, 128], F32, tag="rrf")
                    nc.vector.tensor_tensor(fr[:], y[:], kq[:],
                                            op=ALU.subtract)
                    sn = glue.tile([128, 128], BF16, tag="rrs")
                    nc.scalar.activation(sn[:], fr[:], AF.Sin, scale=TWO_PI)
                    store_feat_row(1 + s, hf, sn[:])
                    # cos(2pi y) = sin(2pi wrap(y + 1/4)): second range
                    # reduction keeps the Sin argument within +-pi
                    y2 = glue.tile([128, 128], F32, tag="rry2")
                    nc.vector.tensor_scalar(y2[:], y[:], 0.25, None,
                                            op0=ALU.add)
                    kq2 = glue.tile([128, 128], F32, tag="rrk2")
                    nc.vector.tensor_scalar(kq2[:], y2[:], MAGIC, -MAGIC,
                                            op0=ALU.add, op1=ALU.add)
                    frc = glue.tile([128, 128], F32, tag="rrfc")
                    nc.vector.tensor_tensor(frc[:], y2[:], kq2[:],
                                            op=ALU.subtract)
                    cs = glue.tile([128, 128], BF16, tag="rrc")
                    nc.scalar.activation(cs[:], frc[:], AF.Sin, scale=TWO_PI)
                    store_feat_row(4 + s, hf, cs[:])
            # read back: strip g <- features of i in [64g, 64g+64)
            for g in range(NQ):
                nc.sync.dma_start(
                    feats2_t[32 * g:32 * g + 7, :],
                    f2d[0:7, g * 64 * 128:(g + 1) * 64 * 128])

            # --- fc2 (out cols 16c..16c+16)
            _emit_kan_layer(tc, feats2_t, d["w1fc2"], e_fc2,
                            bias["b2fc2"][:], skc[0:16, :], GEO_FC2, "fc2",
                            w1_pool=(w1p_fc2_cm, w1p_fc2), w1_pre=fc2_pre,
                            a_bufs=8)
            ep_fc2_cm.__exit__(None, None, None)
            sk_em.w1p_cm.__exit__(None, None, None)
            ep_sk_cm.__exit__(None, None, None)

            # --- AllGather #2: [fc2c; skc] [32,B] -> [2*DOUT, B]
            ag2_in = dpool.tile([32, 128], F32, tag="ag2i")
            ag2_out = dpool.tile([2 * DOUT, 128], F32, tag="ag2o")
            nc.sync.dma_start(ag2_in[:], skc[:])
            nc.gpsimd.collective_compute(
                "AllGather", ALU.bypass, replica_groups=groups,
                ins=[ag2_in[:]], outs=[ag2_out[:]])
            h2t = bigp.tile([128, 128], F32, tag="h2t")
            rest = bigp.tile([128, 128], F32, tag="rest")
            for c in range(NCORES):
                nc.sync.dma_start(h2t[16 * c:16 * c + 16, :],
                                  ag2_out[32 * c:32 * c + 16, :])
                nc.sync.dma_start(rest[16 * c:16 * c + 16, :],
                                  ag2_out[32 * c + 16:32 * c + 32, :])

            # --- GLU gate (replicated, fp32 matmuls)
            gp_cm = tc.tile_pool(name="gpsum", bufs=1,
                                 space=bass.MemorySpace.PSUM)
            gpsum = gp_cm.__enter__()
            z1 = gpsum.tile([128, 128], F32, tag="z1")
            nc.tensor.matmul(z1[:], g1w_t[:], h2t[:], start=True, stop=True)
            z2 = gpsum.tile([128, 128], F32, tag="z2")
            nc.tensor.matmul(z2[:], g2w_t[:], h2t[:], start=True, stop=True)
            sig = glue.tile([128, 128], F32, tag="sig")
            nc.scalar.activation(sig[:], z1[:], AF.Sigmoid, bias=bias["g1b"][:])
            z2b = glue.tile([128, 128], F32, tag="z2b")
            nc.scalar.activation(z2b[:], z2[:], AF.Identity, bias=bias["g2b"][:])
            h3 = glue.tile([128, 128], F32, tag="h3")
            nc.vector.tensor_tensor(h3[:], sig[:], z2b[:], op=ALU.mult)
            h4 = glue.tile([128, 128], F32, tag="h4")
            nc.vector.tensor_tensor(h4[:], h3[:], rest[:], op=ALU.add)

            # --- BatchNorm over batch (free dim) — replicated
            mean = glue.tile([128, 1], F32, tag="mean")
            nc.vector.tensor_reduce(mean[:], h4[:], axis=mybir.AxisListType.X,
                                    op=ALU.add)
            nc.vector.tensor_scalar(mean[:], mean[:], 1.0 / B, None,
                                    op0=ALU.mult)
            sq = glue.tile([128, 128], F32, tag="sq")
            ssq = glue.tile([128, 1], F32, tag="ssq")
            nc.scalar.activation(sq[:], h4[:], AF.Square, accum_out=ssq[:])
            var = glue.tile([128, 1], F32, tag="var")
            nc.vector.tensor_scalar(var[:], ssq[:], 1.0 / B, None,
                                    op0=ALU.mult)
            m2 = glue.tile([128, 1], F32, tag="m2")
            nc.vector.tensor_tensor(m2[:], mean[:], mean[:], op=ALU.mult)
            nc.vector.tensor_tensor(var[:], var[:], m2[:], op=ALU.subtract)
            nc.vector.tensor_scalar(var[:], var[:], BN_EPS, None, op0=ALU.add)
            rec = glue.tile([128, 1], F32, tag="rec")
            nc.vector.reciprocal(rec[:], var[:])
            rstd = glue.tile([128, 1], F32, tag="rstd")
            nc.scalar.activation(rstd[:], rec[:], AF.Sqrt)
            scl = glue.tile([128, 1], F32, tag="scl")
            nc.vector.tensor_tensor(scl[:], rstd[:], bias["bnw"][:],
                                    op=ALU.mult)
            outt = glue.tile([128, 128], F32, tag="outt")
            nc.vector.tensor_scalar(outt[:], h4[:], mean[:], scl[:],
                                    op0=ALU.subtract, op1=ALU.mult)
            nc.vector.tensor_scalar(outt[:], outt[:], bias["bnb"][:], None,
                                    op0=ALU.add)
            nc.sync.dma_start(out_d.ap(), outt[:])
            gp_cm.__exit__(None, None, None)
    nc.compile()
    return nc


_CACHED_NC = None


def kernel(**inputs):
    global _CACHED_NC
    in_maps = prep_in_maps(inputs)
    if _CACHED_NC is None:
        _CACHED_NC = build_program()
    res = bass_utils.run_bass_kernel_spmd(_CACHED_NC, in_maps,
                                          core_ids=list(range(NCORES)))
    out_t = np.asarray(res.results[0]["outT"], np.float32)
    return np.ascontiguousarray(out_t.T)


if __name__ == "__main__":
    rng = np.random.default_rng(0)
    fake = {"x": rng.normal(size=(B, DIN)).astype(np.float32)}
    print("module loads OK")


# revision 49
# speedup vs baseline: 1.1891x; 1.1891x over previous
"""Trainium2 Bass kernel for the 3-layer KAN GRN block.

Sharding: all three KAN layers are sharded over their *output* dim across the
8 cores (fc1: 32 cols/core of DH=256; fc2/sk: 16 cols/core of DOUT=128).
fc2 consumes the full fc1 output via an in-kernel AllGather; a second
AllGather assembles the fc2 + skip outputs, after which the (cheap) GLU gate
and BatchNorm run replicated on every core.

Layout: the input dim of each KAN layer is split into 4 quarters, one per
32-row strip of the PE array (explicit tile_position row tiling).  Stage-1
(the K=8 feature contraction) runs one matmul per (i, k-chunk) with the
weights stationary on strip (i // di4); the four strips execute concurrently
on hardware and, more importantly, every weight/feats DMA now spreads over
4x more SBUF partitions.  silu runs on ACT straight out of PSUM in
[128,1024] tiles; stage-2 (the contraction over hidden H and input dim)
accumulates into a single PSUM tile via matmuls against a block-diagonal W2
that is shipped compact from HBM ([128, din*CH] true bytes) and expanded
on-chip into a persistent SBUF buffer with one small diagonal-fill DMA per
output column (the zero background is written once by memset).
"""

import sys

import numpy as np

for _p in ("/opt/trn_rl_repo",):
    if _p not in sys.path:
        sys.path.insert(0, _p)

import concourse.bass as bass  # noqa: E402
import concourse.tile as tile  # noqa: E402
from concourse import bacc, bass_utils, mybir  # noqa: E402

F32 = mybir.dt.float32
BF16 = mybir.dt.bfloat16
AF = mybir.ActivationFunctionType
ALU = mybir.AluOpType
NP_BF16 = mybir.dt.np(BF16)

ACT_FN = AF.Silu  # patched to AF.Sigmoid for CoreSim structural tests

NCORES = 8
B = 128
DIN = 128
DH = 256
DOUT = 128
H = 32
NSIN = 3
BN_EPS = 1e-5
TWO_PI = float(2.0 * np.pi)
MAGIC = float(1.5 * 2.0**23)  # fp32 round-to-nearest-int trick
NQ = 4  # PE row strips / input-dim quarters
IB = 8  # i values per quarter per weight DMA block

# layer geometry: (din, JL, HC, CH)
GEO_FC1 = (DIN, 32, 4, 8)
GEO_FC2 = (DH, 16, 8, 4)
GEO_SK = (DIN, 16, 8, 4)


def _feats_host(xt):
    """xt: [din, B] f32 -> [32, (din//4)*B] bf16: row (8g+f) = feature f of
    input-quarter g (f7 = const 1 for the B1 fold)."""
    din = xt.shape[0]
    comps = [xt]
    for s in range(NSIN):
        comps.append(np.sin((2.0**s) * xt))
    for s in range(NSIN):
        comps.append(np.cos((2.0**s) * xt))
    comps.append(np.ones_like(xt))
    arr = np.stack(comps, 0)  # [8, din, B]
    di4 = din // NQ
    arr = arr.reshape(8, NQ, di4 * B)  # [f, g, i*B]
    arr = np.transpose(arr, (1, 0, 2)).reshape(32, di4 * B)
    return np.ascontiguousarray(arr.astype(NP_BF16))


def _prep_w1(W1, B1, jsl, geo):
    """-> [32, nblk, IB*CH*128] bf16 lhsT blocks.

    Row (8g+f), block blk, free (isub, k, p=jl*HC+hl) =
      W1e[g*di4 + blk*IB + isub, jl, k*HC+hl, f]."""
    din, JL, HC, CH = geo
    di4 = din // NQ
    nblk = di4 // IB
    W1c = W1[:, jsl].astype(np.float32)  # [din, JL, H, 7]
    B1c = B1[:, jsl].astype(np.float32)  # [din, JL, H]
    W1e = np.concatenate([W1c, B1c[..., None]], axis=-1)  # [din, JL, H, 8]
    t = W1e.reshape(NQ, nblk, IB, JL, CH, HC, 8)  # [g,blk,isub,jl,k,hl,f]
    t = np.transpose(t, (0, 6, 1, 2, 4, 3, 5))  # [g,f,blk,isub,k,jl,hl]
    t = t.reshape(NQ * 8, nblk, IB * CH * 128)
    return np.ascontiguousarray(t.astype(NP_BF16))


def _prep_w2(W2, jsl, geo):
    """-> [128, din*CH] bf16 compact W2: row (jl*HC+hl), col (i*CH+k) =
    W2[i, jl, k*HC+hl]."""
    din, JL, HC, CH = geo
    W2c = W2[:, jsl, 0, :].astype(np.float32)  # [din, JL, H]
    t = W2c.reshape(din, JL, CH, HC)  # [i, jl, k, hl]
    t = np.transpose(t, (1, 3, 0, 2))  # [jl, hl, i, k]
    t = t.reshape(128, din * CH)
    return np.ascontiguousarray(t.astype(NP_BF16))


def _col(v):
    return np.ascontiguousarray(v.astype(np.float32).reshape(-1, 1))


def prep_in_maps(inputs):
    x = np.asarray(inputs["x"], np.float32)
    feats1 = _feats_host(np.ascontiguousarray(x.T))
    in_maps = []
    for c in range(NCORES):
        j1 = slice(32 * c, 32 * c + 32)
        j2 = slice(16 * c, 16 * c + 16)
        m = {
            "feats1": feats1,
            "w1fc1": _prep_w1(inputs["fc1_W1"], inputs["fc1_B1"], j1, GEO_FC1),
            "w2fc1": _prep_w2(inputs["fc1_W2"], j1, GEO_FC1),
            "w1fc2": _prep_w1(inputs["fc2_W1"], inputs["fc2_B1"], j2, GEO_FC2),
            "w2fc2": _prep_w2(inputs["fc2_W2"], j2, GEO_FC2),
            "w1sk": _prep_w1(inputs["sk_W1"], inputs["sk_B1"], j2, GEO_SK),
            "w2sk": _prep_w2(inputs["sk_W2"], j2, GEO_SK),
            "b2fc1": _col(inputs["fc1_B2"][:, j1, 0].sum(0)),
            "b2fc2": _col(inputs["fc2_B2"][:, j2, 0].sum(0)),
            "b2sk": _col(inputs["sk_B2"][:, j2, 0].sum(0)),
            "g1wT": np.ascontiguousarray(
                np.asarray(inputs["g1_w"], np.float32).T),
            "g2wT": np.ascontiguousarray(
                np.asarray(inputs["g2_w"], np.float32).T),
            "g1b": _col(inputs["g1_b"]),
            "g2b": _col(inputs["g2_b"]),
            "bnw": _col(inputs["bn_w"]),
            "bnb": _col(inputs["bn_b"]),
        }
        in_maps.append(m)
    return in_maps


def _emit_w2_expand(nc, e_t, w2_d, geo, split_memset=False):
    """memset e_t to 0 then fill the diagonal blocks from compact DRAM W2."""
    din, JL, HC, CH = geo
    dinCH = din * CH
    ef = e_t[:].bitcast(F32)  # f32 view halves the memset element count
    if split_memset:  # halve the zeroing latency again: DVE + Pool concur
        half = (JL // 2) * dinCH // 2
        nc.vector.memset(ef[:, 0:half], 0.0)
        nc.gpsimd.memset(ef[:, half:], 0.0)
    else:
        nc.gpsimd.memset(ef[:], 0.0)
    for jl in range(JL):
        eng = nc.sync if jl % 2 == 0 else nc.gpsimd
        eng.dma_start(
            e_t[jl * HC:(jl + 1) * HC, jl * dinCH:(jl + 1) * dinCH],
            w2_d.ap()[jl * HC:(jl + 1) * HC, :])


def _emit_w1_block(nc, w1p, w1_d, geo, blk):
    """DMA one W1 block (4 strip transfers) into a fresh pool tile."""
    din, JL, HC, CH = geo
    w1t = w1p.tile([128, IB * CH * 128], BF16, tag="w1")
    for g in range(NQ):
        nc.sync.dma_start(w1t[32 * g:32 * g + 8, :],
                          w1_d.ap()[8 * g:8 * g + 8, blk])
    return w1t


class _KanEmitter:
    """Streaming emitter for one KAN layer; blocks may be emitted in
    several groups (interleaved with other program phases).

    feats_t: [128, di4*B] bf16 (strip g rows 32g..32g+8 = features of
    quarter g).  w1_d: DRAM [32, nblk, IB*CH*128].  e_t: expanded W2
    [128, JL*din*CH] bf16 SBUF tile."""

    def __init__(self, tc, feats_t, w1_d, e_t, geo, name,
                 w1_pool=None):
        self.tc, self.nc = tc, tc.nc
        self.feats_t, self.w1_d, self.geo, self.name = feats_t, w1_d, geo, name
        din, JL, HC, CH = geo
        self.di4 = din // NQ
        self.nblk = self.di4 // IB
        self.kp = 2
        self.npacks = CH // self.kp
        self.ev = e_t[:].rearrange("p (j c) -> p c j", j=JL)
        self.n_acc = din * CH
        self.acc = 0
        if w1_pool is None:
            self.w1p_cm = tc.tile_pool(name=f"{name}_w1", bufs=2)
            self.w1p = self.w1p_cm.__enter__()
        else:
            self.w1p_cm, self.w1p = w1_pool
        self.o_cm = tc.tile_pool(name=f"{name}_o", bufs=1,
                                 space=bass.MemorySpace.PSUM)
        self.out2 = self.o_cm.__enter__().tile([JL, 128], F32)
        self.w1_pre = {}

    def prefetch(self, blocks):
        for blk in blocks:
            self.w1_pre[blk] = _emit_w1_block(self.nc, self.w1p, self.w1_d,
                                              self.geo, blk)

    def emit_blocks(self, blocks, a_bufs=6):
        nc, tc = self.nc, self.tc
        din, JL, HC, CH = self.geo
        kp, name = self.kp, self.name
        with tc.tile_pool(name=f"{name}_h", bufs=3,
                          space=bass.MemorySpace.PSUM) as hp_pool, \
             tc.tile_pool(name=f"{name}_a", bufs=a_bufs) as a_pool:
            ipp = 8 // CH  # isub's per pack (pack = 8 chunk slots, 1 strip)
            for blk in blocks:
                w1t = self.w1_pre.pop(blk, None)
                if w1t is None:
                    w1t = _emit_w1_block(nc, self.w1p, self.w1_d, self.geo,
                                         blk)
                for ig in range(IB // ipp):
                    for g in range(NQ):
                        # Whole pack on ONE PE row strip; consecutive packs
                        # rotate strips.  Mixing strips within the in-flight
                        # window while a matmul blocks on a semaphore crashes
                        # the PE (HW-verified), so strips only change at
                        # pack boundaries.
                        hp = hp_pool.tile([128, 1024], F32, tag="h")
                        for i2 in range(ipp):
                            isub = ig * ipp + i2
                            islot = blk * IB + isub
                            for k in range(CH):
                                s = i2 * CH + k
                                nc.tensor.matmul(
                                    hp[:, s * 128:(s + 1) * 128],
                                    w1t[32 * g:32 * g + 8,
                                        (isub * CH + k) * 128:
                                        (isub * CH + k + 1) * 128],
                                    self.feats_t[32 * g:32 * g + 8,
                                                 islot * 128:
                                                 (islot + 1) * 128],
                                    start=True, stop=True,
                                    tile_position=(32 * g, 0))
                        at = a_pool.tile([128, 1024], BF16, tag="a")
                        nc.scalar.activation(at[:], hp[:], ACT_FN)
                        for i2 in range(ipp):
                            isub = ig * ipp + i2
                            i = g * self.di4 + blk * IB + isub
                            for k in range(CH):
                                s = i2 * CH + k
                                nc.tensor.matmul(
                                    self.out2[:],
                                    self.ev[:, i * CH + k, :],
                                    at[:, s * 128:(s + 1) * 128],
                                    start=(self.acc == 0),
                                    stop=(self.acc == self.n_acc - 1))
                                self.acc += 1

    def finish(self, b2_t, out_sb, act_hook=None, close_w1=True):
        assert self.acc == self.n_acc
        if act_hook is None:
            self.nc.vector.tensor_scalar(out_sb, self.out2[:], b2_t, None,
                                         op0=ALU.add)
        else:
            act_hook(self.out2, b2_t, out_sb)
        self.o_cm.__exit__(None, None, None)
        if close_w1:
            self.w1p_cm.__exit__(None, None, None)


def _emit_kan_layer(tc, feats_t, w1_d, e_t, b2_t, out_sb, geo, name,
                    act_hook=None, prefetch=(), w1_pool=None, w1_pre=None,
                    a_bufs=6):
    em = _KanEmitter(tc, feats_t, w1_d, e_t, geo, name, w1_pool=w1_pool)
    if w1_pre:
        em.w1_pre = dict(w1_pre)
    if prefetch:
        em.prefetch(prefetch)
    em.emit_blocks(range(em.nblk), a_bufs=a_bufs)
    em.finish(b2_t, out_sb, act_hook)


def build_program():
    nc = bacc.Bacc("TRN2", target_bir_lowering=False, debug=False,
                   num_devices=NCORES)
    d = {}
    d["feats1"] = nc.dram_tensor("feats1", [32, (DIN // NQ) * B], BF16,
                                 kind="ExternalInput")
    d["w1fc1"] = nc.dram_tensor("w1fc1", [32, (DIN // NQ) // IB, IB * 8 * 128],
                                BF16, kind="ExternalInput")
    d["w2fc1"] = nc.dram_tensor("w2fc1", [128, DIN * 8], BF16,
                                kind="ExternalInput")
    d["w1fc2"] = nc.dram_tensor("w1fc2", [32, (DH // NQ) // IB, IB * 4 * 128],
                                BF16, kind="ExternalInput")
    d["w2fc2"] = nc.dram_tensor("w2fc2", [128, DH * 4], BF16,
                                kind="ExternalInput")
    d["w1sk"] = nc.dram_tensor("w1sk", [32, (DIN // NQ) // IB, IB * 4 * 128],
                               BF16, kind="ExternalInput")
    d["w2sk"] = nc.dram_tensor("w2sk", [128, DIN * 4], BF16,
                               kind="ExternalInput")
    for nm, rows in (("b2fc1", 32), ("b2fc2", 16), ("b2sk", 16), ("g1b", 128),
                     ("g2b", 128), ("bnw", 128), ("bnb", 128)):
        d[nm] = nc.dram_tensor(nm, [rows, 1], F32, kind="ExternalInput")
    d["g1wT"] = nc.dram_tensor("g1wT", [128, 128], F32, kind="ExternalInput")
    d["g2wT"] = nc.dram_tensor("g2wT", [128, 128], F32, kind="ExternalInput")
    out_d = nc.dram_tensor("outT", [DOUT, B], F32, kind="ExternalOutput")

    groups = [list(range(NCORES))]

    with tile.TileContext(nc) as tc:
        with tc.tile_pool(name="const", bufs=1) as cpool, \
             tc.tile_pool(name="feats", bufs=1) as fpool, \
             tc.tile_pool(name="big", bufs=1) as bigp, \
             tc.tile_pool(name="dram", bufs=2,
                          space=bass.MemorySpace.DRAM) as dpool, \
             tc.tile_pool(name="glue", bufs=2) as glue:
            # --- feats for fc1/sk (host-computed), 4 strip DMAs
            feats1_t = fpool.tile([128, (DIN // NQ) * B], BF16, tag="f1")
            for g in range(NQ):
                nc.sync.dma_start(feats1_t[32 * g:32 * g + 8, :],
                                  d["feats1"].ap()[8 * g:8 * g + 8, :])

            # --- sk emitter: first half runs before fc1 (covers the E1
            # expansion), second half after AG1 (covers the collective)
            ep_sk_cm = tc.tile_pool(name="w2e_sk", bufs=1)
            e_sk = ep_sk_cm.__enter__().tile([128, 16 * DIN * 4], BF16,
                                             tag="esk")
            sk_em = _KanEmitter(tc, feats1_t, d["w1sk"], e_sk, GEO_SK, "sk")
            sk_em.prefetch([0, 1])
            _emit_w2_expand(nc, e_sk, d["w2sk"], GEO_SK)

            # --- prefetch fc1's first two W1 blocks BEFORE the E1 fills so
            # stage-1 is never queued behind them on SP
            fc1_em = None  # created after prefetches to keep SP queue clean
            w1p_fc1_cm = tc.tile_pool(name="fc1_w1", bufs=2)
            w1p_fc1 = w1p_fc1_cm.__enter__()
            fc1_pre = {blk: _emit_w1_block(nc, w1p_fc1, d["w1fc1"], GEO_FC1,
                                           blk) for blk in range(2)}

            # --- expanded block-diagonal W2 for fc1
            ep_fc1_cm = tc.tile_pool(name="w2e_fc1", bufs=1)
            e_fc1 = ep_fc1_cm.__enter__().tile([128, 32 * DIN * 8], BF16,
                                               tag="efc1")
            _emit_w2_expand(nc, e_fc1, d["w2fc1"], GEO_FC1, split_memset=True)

            # --- constants (all consumed at layer tails — emit after the
            # start-critical DMA streams so they never delay them)
            bias = {}
            for nm, rows in (("b2fc1", 32), ("b2fc2", 16), ("b2sk", 16),
                             ("g1b", 128), ("g2b", 128), ("bnw", 128),
                             ("bnb", 128)):
                t = cpool.tile([rows, 1], F32, tag=nm)
                nc.sync.dma_start(t[:], d[nm].ap())
                bias[nm] = t
            g1w_t = cpool.tile([128, 128], F32, tag="g1w")
            nc.sync.dma_start(g1w_t[:], d["g1wT"].ap())
            g2w_t = cpool.tile([128, 128], F32, tag="g2w")
            nc.sync.dma_start(g2w_t[:], d["g2wT"].ap())


            # --- sk first half (covers the E1 build latency)
            sk_em.emit_blocks([0, 1], a_bufs=12)

            # --- fc1 (out cols 32c..32c+32), elu fused into output hook
            h1c = bigp.tile([32, 128], F32, tag="h1c")

            def elu_hook(out2, b2_t, out_sb):
                z = glue.tile([32, 128], F32, tag="eluz")
                nc.vector.tensor_scalar(z[:], out2[:], b2_t, None, op0=ALU.add)
                zn = glue.tile([32, 128], F32, tag="elun")
                nc.vector.tensor_scalar(zn[:], z[:], 0.0, None, op0=ALU.min)
                e = glue.tile([32, 128], F32, tag="elue")
                nc.scalar.activation(e[:], zn[:], AF.Exp)
                r = glue.tile([32, 128], F32, tag="elur")
                nc.vector.tensor_scalar(r[:], z[:], 0.0, None, op0=ALU.max)
                nc.vector.scalar_tensor_tensor(
                    out_sb, e[:], -1.0, r[:], op0=ALU.add, op1=ALU.add)

            fc1_em = _KanEmitter(tc, feats1_t, d["w1fc1"], e_fc1, GEO_FC1,
                                 "fc1", w1_pool=(w1p_fc1_cm, w1p_fc1))
            fc1_em.w1_pre = fc1_pre
            fc1_em.emit_blocks(range(fc1_em.nblk))
            fc1_em.finish(bias["b2fc1"][:], h1c[:], act_hook=elu_hook,
                          close_w1=False)
            ep_fc1_cm.__exit__(None, None, None)  # e_fc1 dead; reuse space
            w1p_fc1_cm.__exit__(None, None, None)

            # --- expand fc2's W2 (fills run under AG1 / sk second half),
            # and prefetch fc2's first W1 blocks in the same window
            ep_fc2_cm = tc.tile_pool(name="w2e_fc2", bufs=1)
            e_fc2 = ep_fc2_cm.__enter__().tile([128, 16 * DH * 4], BF16,
                                               tag="efc2")
            _emit_w2_expand(nc, e_fc2, d["w2fc2"], GEO_FC2)
            w1p_fc2_cm = tc.tile_pool(name="fc2_w1", bufs=2)
            w1p_fc2 = w1p_fc2_cm.__enter__()
            fc2_pre = {blk: _emit_w1_block(nc, w1p_fc2, d["w1fc2"], GEO_FC2,
                                           blk) for blk in range(2)}

            # --- AllGather #1: h1c [32,B] -> h1 [DH,B]
            ag1_in = dpool.tile([32, 128], F32, tag="ag1i")
            ag1_out = dpool.tile([DH, 128], F32, tag="ag1o")
            nc.sync.dma_start(ag1_in[:], h1c[:])
            nc.gpsimd.collective_compute(
                "AllGather", ALU.bypass, replica_groups=groups,
                ins=[ag1_in[:]], outs=[ag1_out[:]])

            # --- sk second half (covers the AllGather + trig chain)
            skc = bigp.tile([32, 128], F32, tag="catc")
            sk_tmp = bigp.tile([16, 128], F32, tag="sktmp")
            sk_em.emit_blocks([2, 3])
            sk_em.finish(bias["b2sk"][:], sk_tmp[:], close_w1=False)
            nc.sync.dma_start(skc[16:32, :], sk_tmp[:])

            # --- fc2 feature build (on-device trig with range reduction)
            # h1t: AG1 result as 2 tiles of [128(i), 128(b)] f32
            h1t = [bigp.tile([128, 128], F32, tag=f"h1t{hf}", name=f"h1t{hf}")
                   for hf in range(2)]
            for hf in range(2):
                nc.sync.dma_start(h1t[hf][:],
                                  ag1_out[128 * hf:128 * hf + 128, :])
            # feats2 strip g rows 32g+f = feature f of i in [64g, 64g+64)
            feats2_t = fpool.tile([128, (DH // NQ) * B], BF16, tag="f2")
            nc.gpsimd.memset(feats2_t[:], 1.0)  # const-1 row background
            # bounce DRAM tensor: row f = feature f, free (i in 256, b);
            # only 7 feature rows — the const-1 row stays from the memset
            f2d = dpool.tile([7, DH * B], BF16, tag="f2d")

            def store_feat_row(s, hf, src_bf16):
                # src [128(i), 128(b)] -> f2d[s, hf*16384 + i*128 + b]
                dst = f2d[s:s + 1, hf * 16384:(hf + 1) * 16384]
                vt = dst.rearrange("one (p b) -> (one p) b", p=128)
                nc.sync.dma_start(vt, src_bf16)

            for hf in range(2):
                c0 = glue.tile([128, 128], BF16, tag="c0")
                nc.vector.tensor_copy(c0[:], h1t[hf][:])
                store_feat_row(0, hf, c0[:])
                for s in range(NSIN):
                    sc = (2.0**s) / TWO_PI
                    y = glue.tile([128, 128], F32, tag="rry")
                    nc.vector.tensor_scalar(y[:], h1t[hf][:], sc, None,
                                            op0=ALU.mult)
                    kq = glue.tile([128, 128], F32, tag="rrk")
                    nc.vector.tensor_scalar(kq[:], y[:], MAGIC, -MAGIC,
                                            op0=ALU.add, op1=ALU.add)
                    fr = glue.tile([128, 128], F32, tag="rrf")
                    nc.vector.tensor_tensor(fr[:], y[:], kq[:],
                                            op=ALU.subtract)
                    sn = glue.tile([128, 128], BF16, tag="rrs")
                    nc.scalar.activation(sn[:], fr[:], AF.Sin, scale=TWO_PI)
                    store_feat_row(1 + s, hf, sn[:])
                    # cos(2pi y) = sin(2pi wrap(y + 1/4)): second range
                    # reduction keeps the Sin argument within +-pi
                    y2 = glue.tile([128, 128], F32, tag="rry2")
                    nc.vector.tensor_scalar(y2[:], y[:], 0.25, None,
                                            op0=ALU.add)
                    kq2 = glue.tile([128, 128], F32, tag="rrk2")
                    nc.vector.tensor_scalar(kq2[:], y2[:], MAGIC, -MAGIC,
                                            op0=ALU.add, op1=ALU.add)
                    frc = glue.tile([128, 128], F32, tag="rrfc")
                    nc.vector.tensor_tensor(frc[:], y2[:], kq2[:],
                                            op=ALU.subtract)
                    cs = glue.tile([128, 128], BF16, tag="rrc")
                    nc.scalar.activation(cs[:], frc[:], AF.Sin, scale=TWO_PI)
                    store_feat_row(4 + s, hf, cs[:])
            # read back: strip g <- features of i in [64g, 64g+64)
            for g in range(NQ):
                nc.sync.dma_start(
                    feats2_t[32 * g:32 * g + 7, :],
                    f2d[0:7, g * 64 * 128:(g + 1) * 64 * 128])

            # --- fc2 (out cols 16c..16c+16)
            _emit_kan_layer(tc, feats2_t, d["w1fc2"], e_fc2,
                            bias["b2fc2"][:], skc[0:16, :], GEO_FC2, "fc2",
                            w1_pool=(w1p_fc2_cm, w1p_fc2), w1_pre=fc2_pre)
            ep_fc2_cm.__exit__(None, None, None)
            sk_em.w1p_cm.__exit__(None, None, None)
            ep_sk_cm.__exit__(None, None, None)

            # --- AllGather #2: [fc2c; skc] [32,B] -> [2*DOUT, B]
            ag2_in = dpool.tile([32, 128], F32, tag="ag2i")
            ag2_out = dpool.tile([2 * DOUT, 128], F32, tag="ag2o")
            nc.sync.dma_start(ag2_in[:], skc[:])
            nc.gpsimd.collective_compute(
                "AllGather", ALU.bypass, replica_groups=groups,
                ins=[ag2_in[:]], outs=[ag2_out[:]])
            h2t = bigp.tile([128, 128], F32, tag="h2t")
            rest = bigp.tile([128, 128], F32, tag="rest")
            for c in range(NCORES):
                nc.sync.dma_start(h2t[16 * c:16 * c + 16, :],
                                  ag2_out[32 * c:32 * c + 16, :])
                nc.sync.dma_start(rest[16 * c:16 * c + 16, :],
                                  ag2_out[32 * c + 16:32 * c + 32, :])

            # --- GLU gate (replicated, fp32 matmuls)
            gp_cm = tc.tile_pool(name="gpsum", bufs=1,
                                 space=bass.MemorySpace.PSUM)
            gpsum = gp_cm.__enter__()
            z1 = gpsum.tile([128, 128], F32, tag="z1")
            nc.tensor.matmul(z1[:], g1w_t[:], h2t[:], start=True, stop=True)
            z2 = gpsum.tile([128, 128], F32, tag="z2")
            nc.tensor.matmul(z2[:], g2w_t[:], h2t[:], start=True, stop=True)
            sig = glue.tile([128, 128], F32, tag="sig")
            nc.scalar.activation(sig[:], z1[:], AF.Sigmoid, bias=bias["g1b"][:])
            z2b = glue.tile([128, 128], F32, tag="z2b")
            nc.scalar.activation(z2b[:], z2[:], AF.Identity, bias=bias["g2b"][:])
            h3 = glue.tile([128, 128], F32, tag="h3")
            nc.vector.tensor_tensor(h3[:], sig[:], z2b[:], op=ALU.mult)
            h4 = glue.tile([128, 128], F32, tag="h4")
            nc.vector.tensor_tensor(h4[:], h3[:], rest[:], op=ALU.add)

            # --- BatchNorm over batch (free dim) — replicated
            mean = glue.tile([128, 1], F32, tag="mean")
            nc.vector.tensor_reduce(mean[:], h4[:], axis=mybir.AxisListType.X,
                                    op=ALU.add)
            nc.vector.tensor_scalar(mean[:], mean[:], 1.0 / B, None,
                                    op0=ALU.mult)
            sq = glue.tile([128, 128], F32, tag="sq")
            ssq = glue.tile([128, 1], F32, tag="ssq")
            nc.scalar.activation(sq[:], h4[:], AF.Square, accum_out=ssq[:])
            var = glue.tile([128, 1], F32, tag="var")
            nc.vector.tensor_scalar(var[:], ssq[:], 1.0 / B, None,
                                    op0=ALU.mult)
            m2 = glue.tile([128, 1], F32, tag="m2")
            nc.vector.tensor_tensor(m2[:], mean[:], mean[:], op=ALU.mult)
            nc.vector.tensor_tensor(var[:], var[:], m2[:], op=ALU.subtract)
            nc.vector.tensor_scalar(var[:], var[:], BN_EPS, None, op0=ALU.add)
            rec = glue.tile([128, 1], F32, tag="rec")
            nc.vector.reciprocal(rec[:], var[:])
            rstd = glue.tile([128, 1], F32, tag="rstd")
            nc.scalar.activation(rstd[:], rec[:], AF.Sqrt)
            scl = glue.tile([128, 1], F32, tag="scl")
            nc.vector.tensor_tensor(scl[:], rstd[:], bias["bnw"][:],
                                    op=ALU.mult)
            outt = glue.tile([128, 128], F32, tag="outt")
            nc.vector.tensor_scalar(outt[:], h4[:], mean[:], scl[:],
                                    op0=ALU.subtract, op1=ALU.mult)
            nc.vector.tensor_scalar(outt[:], outt[:], bias["bnb"][:], None,
                                    op0=ALU.add)
            nc.sync.dma_start(out_d.ap(), outt[:])
            gp_cm.__exit__(None, None, None)
    nc.compile()
    return nc


_CACHED_NC = None


def kernel(**inputs):
    global _CACHED_NC
    in_maps = prep_in_maps(inputs)
    if _CACHED_NC is None:
        _CACHED_NC = build_program()
    res = bass_utils.run_bass_kernel_spmd(_CACHED_NC, in_maps,
                                          core_ids=list(range(NCORES)))
    out_t = np.asarray(res.results[0]["outT"], np.float32)
    return np.ascontiguousarray(out_t.T)


if __name__ == "__main__":
    rng = np.random.default_rng(0)
    fake = {"x": rng.normal(size=(B, DIN)).astype(np.float32)}
    print("module loads OK")


# revision 58
# speedup vs baseline: 1.3713x; 1.1532x over previous
"""Trainium2 Bass kernel for the 3-layer KAN GRN block.

Sharding: all three KAN layers are sharded over their *output* dim across the
8 cores (fc1: 32 cols/core of DH=256; fc2/sk: 16 cols/core of DOUT=128).
fc2 consumes the full fc1 output via an in-kernel AllGather; a second
AllGather assembles the fc2 + skip outputs, after which the (cheap) GLU gate
and BatchNorm run replicated on every core.

Layout: the input dim of each KAN layer is split into 4 quarters, one per
32-row strip of the PE array (explicit tile_position row tiling).  Stage-1
(the K=8 feature contraction) runs one matmul per (i, k-chunk) with the
weights stationary on strip (i // di4); the four strips execute concurrently
on hardware and, more importantly, every weight/feats DMA now spreads over
4x more SBUF partitions.  silu runs on ACT straight out of PSUM in
[128,1024] tiles; stage-2 (the contraction over hidden H and input dim)
accumulates into a single PSUM tile via matmuls against a block-diagonal W2
that is shipped compact from HBM ([128, din*CH] true bytes) and expanded
on-chip into a persistent SBUF buffer with one small diagonal-fill DMA per
output column (the zero background is written once by memset).
"""

import sys

import numpy as np

for _p in ("/opt/trn_rl_repo",):
    if _p not in sys.path:
        sys.path.insert(0, _p)

import concourse.bass as bass  # noqa: E402
import concourse.tile as tile  # noqa: E402
from concourse import bacc, bass_utils, mybir  # noqa: E402

F32 = mybir.dt.float32
BF16 = mybir.dt.bfloat16
AF = mybir.ActivationFunctionType
ALU = mybir.AluOpType
NP_BF16 = mybir.dt.np(BF16)

ACT_FN = AF.Silu  # patched to AF.Sigmoid for CoreSim structural tests

NCORES = 8
B = 128
DIN = 128
DH = 256
DOUT = 128
H = 32
NSIN = 3
BN_EPS = 1e-5
TWO_PI = float(2.0 * np.pi)
MAGIC = float(1.5 * 2.0**23)  # fp32 round-to-nearest-int trick
NQ = 4  # PE row strips / input-dim quarters
IB = 8  # i values per quarter per weight DMA block

# layer geometry: (din, JL, HC, CH)
GEO_FC1 = (DIN, 32, 4, 8)
GEO_FC2 = (DH, 16, 8, 4)
GEO_SK = (DIN, 16, 8, 4)


def _feats_host(xt):
    """xt: [din, B] f32 -> [32, (din//4)*B] bf16: row (8g+f) = feature f of
    input-quarter g (f7 = const 1 for the B1 fold)."""
    din = xt.shape[0]
    comps = [xt]
    for s in range(NSIN):
        comps.append(np.sin((2.0**s) * xt))
    for s in range(NSIN):
        comps.append(np.cos((2.0**s) * xt))
    comps.append(np.ones_like(xt))
    arr = np.stack(comps, 0)  # [8, din, B]
    di4 = din // NQ
    arr = arr.reshape(8, NQ, di4 * B)  # [f, g, i*B]
    arr = np.transpose(arr, (1, 0, 2)).reshape(32, di4 * B)
    return np.ascontiguousarray(arr.astype(NP_BF16))


def _prep_w1(W1, B1, jsl, geo):
    """-> [32, nblk, IB*CH*128] bf16 lhsT blocks.

    Row (8g+f), block blk, free (isub, k, p=jl*HC+hl) =
      W1e[g*di4 + blk*IB + isub, jl, k*HC+hl, f]."""
    din, JL, HC, CH = geo
    di4 = din // NQ
    nblk = di4 // IB
    W1c = W1[:, jsl].astype(np.float32)  # [din, JL, H, 7]
    B1c = B1[:, jsl].astype(np.float32)  # [din, JL, H]
    W1e = np.concatenate([W1c, B1c[..., None]], axis=-1)  # [din, JL, H, 8]
    t = W1e.reshape(NQ, nblk, IB, JL, CH, HC, 8)  # [g,blk,isub,jl,k,hl,f]
    t = np.transpose(t, (0, 6, 1, 2, 4, 3, 5))  # [g,f,blk,isub,k,jl,hl]
    t = t.reshape(NQ * 8, nblk, IB * CH * 128)
    return np.ascontiguousarray(t.astype(NP_BF16))


def _prep_w2(W2, jsl, geo):
    """-> [128, din*CH] bf16 compact W2: row (jl*HC+hl), col (i*CH+k) =
    W2[i, jl, k*HC+hl]."""
    din, JL, HC, CH = geo
    W2c = W2[:, jsl, 0, :].astype(np.float32)  # [din, JL, H]
    t = W2c.reshape(din, JL, CH, HC)  # [i, jl, k, hl]
    t = np.transpose(t, (1, 3, 0, 2))  # [jl, hl, i, k]
    t = t.reshape(128, din * CH)
    return np.ascontiguousarray(t.astype(NP_BF16))


def _col(v):
    return np.ascontiguousarray(v.astype(np.float32).reshape(-1, 1))


def prep_in_maps(inputs):
    x = np.asarray(inputs["x"], np.float32)
    feats1 = _feats_host(np.ascontiguousarray(x.T))
    in_maps = []
    for c in range(NCORES):
        j1 = slice(32 * c, 32 * c + 32)
        j2 = slice(16 * c, 16 * c + 16)
        m = {
            "feats1": feats1,
            "w1fc1": _prep_w1(inputs["fc1_W1"], inputs["fc1_B1"], j1, GEO_FC1),
            "w2fc1": _prep_w2(inputs["fc1_W2"], j1, GEO_FC1),
            "w1fc2": _prep_w1(inputs["fc2_W1"], inputs["fc2_B1"], j2, GEO_FC2),
            "w2fc2": _prep_w2(inputs["fc2_W2"], j2, GEO_FC2),
            "w1sk": _prep_w1(inputs["sk_W1"], inputs["sk_B1"], j2, GEO_SK),
            "w2sk": _prep_w2(inputs["sk_W2"], j2, GEO_SK),
            "b2fc1": _col(inputs["fc1_B2"][:, j1, 0].sum(0)),
            "b2fc2": _col(inputs["fc2_B2"][:, j2, 0].sum(0)),
            "b2sk": _col(inputs["sk_B2"][:, j2, 0].sum(0)),
            "g1wT": np.ascontiguousarray(
                np.asarray(inputs["g1_w"], np.float32).T),
            "g2wT": np.ascontiguousarray(
                np.asarray(inputs["g2_w"], np.float32).T),
            "g1b": _col(inputs["g1_b"]),
            "g2b": _col(inputs["g2_b"]),
            "bnw": _col(inputs["bn_w"]),
            "bnb": _col(inputs["bn_b"]),
        }
        in_maps.append(m)
    return in_maps


def _emit_w2_expand(nc, e_t, w2_d, geo, split_memset=False):
    """memset e_t to 0 then fill the diagonal blocks from compact DRAM W2."""
    din, JL, HC, CH = geo
    dinCH = din * CH
    ef = e_t[:].bitcast(F32)  # f32 view halves the memset element count
    if split_memset:  # halve the zeroing latency again: DVE + Pool concur
        half = (JL // 2) * dinCH // 2
        nc.vector.memset(ef[:, 0:half], 0.0)
        nc.gpsimd.memset(ef[:, half:], 0.0)
    else:
        nc.gpsimd.memset(ef[:], 0.0)
    for jl in range(JL):
        eng = nc.sync if jl % 2 == 0 else nc.gpsimd
        eng.dma_start(
            e_t[jl * HC:(jl + 1) * HC, jl * dinCH:(jl + 1) * dinCH],
            w2_d.ap()[jl * HC:(jl + 1) * HC, :])


def _emit_w1_block(nc, w1p, w1_d, geo, blk):
    """DMA one W1 block (4 strip transfers) into a fresh pool tile."""
    din, JL, HC, CH = geo
    w1t = w1p.tile([128, IB * CH * 128], BF16, tag="w1")
    for g in range(NQ):
        nc.sync.dma_start(w1t[32 * g:32 * g + 8, :],
                          w1_d.ap()[8 * g:8 * g + 8, blk])
    return w1t


class _KanEmitter:
    """Streaming emitter for one KAN layer; blocks may be emitted in
    several groups (interleaved with other program phases).

    feats_t: [128, di4*B] bf16 (strip g rows 32g..32g+8 = features of
    quarter g).  w1_d: DRAM [32, nblk, IB*CH*128].  e_t: expanded W2
    [128, JL*din*CH] bf16 SBUF tile."""

    def __init__(self, tc, feats_t, w1_d, e_t, geo, name,
                 w1_pool=None):
        self.tc, self.nc = tc, tc.nc
        self.feats_t, self.w1_d, self.geo, self.name = feats_t, w1_d, geo, name
        din, JL, HC, CH = geo
        self.di4 = din // NQ
        self.nblk = self.di4 // IB
        self.kp = 2
        self.npacks = CH // self.kp
        self.ev = e_t[:].rearrange("p (j c) -> p c j", j=JL)
        self.n_acc = din * CH
        self.acc = 0
        if w1_pool is None:
            self.w1p_cm = tc.tile_pool(name=f"{name}_w1", bufs=2)
            self.w1p = self.w1p_cm.__enter__()
        else:
            self.w1p_cm, self.w1p = w1_pool
        self.o_cm = tc.tile_pool(name=f"{name}_o", bufs=1,
                                 space=bass.MemorySpace.PSUM)
        self.out2 = self.o_cm.__enter__().tile([JL, 128], F32)
        self.w1_pre = {}

    def prefetch(self, blocks):
        for blk in blocks:
            self.w1_pre[blk] = _emit_w1_block(self.nc, self.w1p, self.w1_d,
                                              self.geo, blk)

    def emit_blocks(self, blocks, a_bufs=6):
        nc, tc = self.nc, self.tc
        din, JL, HC, CH = self.geo
        kp, name = self.kp, self.name
        with tc.tile_pool(name=f"{name}_h", bufs=3,
                          space=bass.MemorySpace.PSUM) as hp_pool, \
             tc.tile_pool(name=f"{name}_a", bufs=a_bufs) as a_pool:
            ipp = 8 // CH  # isub's per pack (pack = 8 chunk slots, 1 strip)
            for blk in blocks:
                w1t = self.w1_pre.pop(blk, None)
                if w1t is None:
                    w1t = _emit_w1_block(nc, self.w1p, self.w1_d, self.geo,
                                         blk)
                for ig in range(IB // ipp):
                    for g in range(NQ):
                        # Whole pack on ONE PE row strip; consecutive packs
                        # rotate strips.  Mixing strips within the in-flight
                        # window while a matmul blocks on a semaphore crashes
                        # the PE (HW-verified), so strips only change at
                        # pack boundaries.
                        hp = hp_pool.tile([128, 1024], F32, tag="h")
                        for i2 in range(ipp):
                            isub = ig * ipp + i2
                            islot = blk * IB + isub
                            for k in range(CH):
                                s = i2 * CH + k
                                nc.tensor.matmul(
                                    hp[:, s * 128:(s + 1) * 128],
                                    w1t[32 * g:32 * g + 8,
                                        (isub * CH + k) * 128:
                                        (isub * CH + k + 1) * 128],
                                    self.feats_t[32 * g:32 * g + 8,
                                                 islot * 128:
                                                 (islot + 1) * 128],
                                    start=True, stop=True,
                                    tile_position=(32 * g, 0))
                        at = a_pool.tile([128, 1024], BF16, tag="a")
                        nc.scalar.activation(at[:], hp[:], ACT_FN)
                        for i2 in range(ipp):
                            isub = ig * ipp + i2
                            i = g * self.di4 + blk * IB + isub
                            for k in range(CH):
                                s = i2 * CH + k
                                nc.tensor.matmul(
                                    self.out2[:],
                                    self.ev[:, i * CH + k, :],
                                    at[:, s * 128:(s + 1) * 128],
                                    start=(self.acc == 0),
                                    stop=(self.acc == self.n_acc - 1))
                                self.acc += 1

    def finish(self, b2_t, out_sb, act_hook=None, close_w1=True):
        assert self.acc == self.n_acc
        if act_hook is None:
            self.nc.vector.tensor_scalar(out_sb, self.out2[:], b2_t, None,
                                         op0=ALU.add)
        else:
            act_hook(self.out2, b2_t, out_sb)
        self.o_cm.__exit__(None, None, None)
        if close_w1:
            self.w1p_cm.__exit__(None, None, None)


def _emit_kan_layer(tc, feats_t, w1_d, e_t, b2_t, out_sb, geo, name,
                    act_hook=None, prefetch=(), w1_pool=None, w1_pre=None,
                    a_bufs=6):
    em = _KanEmitter(tc, feats_t, w1_d, e_t, geo, name, w1_pool=w1_pool)
    if w1_pre:
        em.w1_pre = dict(w1_pre)
    if prefetch:
        em.prefetch(prefetch)
    em.emit_blocks(range(em.nblk), a_bufs=a_bufs)
    em.finish(b2_t, out_sb, act_hook)


def build_program():
    nc = bacc.Bacc("TRN2", target_bir_lowering=False, debug=False,
                   num_devices=NCORES)
    d = {}
    d["feats1"] = nc.dram_tensor("feats1", [32, (DIN // NQ) * B], BF16,
                                 kind="ExternalInput")
    d["w1fc1"] = nc.dram_tensor("w1fc1", [32, (DIN // NQ) // IB, IB * 8 * 128],
                                BF16, kind="ExternalInput")
    d["w2fc1"] = nc.dram_tensor("w2fc1", [128, DIN * 8], BF16,
                                kind="ExternalInput")
    d["w1fc2"] = nc.dram_tensor("w1fc2", [32, (DH // NQ) // IB, IB * 4 * 128],
                                BF16, kind="ExternalInput")
    d["w2fc2"] = nc.dram_tensor("w2fc2", [128, DH * 4], BF16,
                                kind="ExternalInput")
    d["w1sk"] = nc.dram_tensor("w1sk", [32, (DIN // NQ) // IB, IB * 4 * 128],
                               BF16, kind="ExternalInput")
    d["w2sk"] = nc.dram_tensor("w2sk", [128, DIN * 4], BF16,
                               kind="ExternalInput")
    for nm, rows in (("b2fc1", 32), ("b2fc2", 16), ("b2sk", 16), ("g1b", 128),
                     ("g2b", 128), ("bnw", 128), ("bnb", 128)):
        d[nm] = nc.dram_tensor(nm, [rows, 1], F32, kind="ExternalInput")
    d["g1wT"] = nc.dram_tensor("g1wT", [128, 128], F32, kind="ExternalInput")
    d["g2wT"] = nc.dram_tensor("g2wT", [128, 128], F32, kind="ExternalInput")
    out_d = nc.dram_tensor("outT", [DOUT, B], F32, kind="ExternalOutput")

    groups = [list(range(NCORES))]

    with tile.TileContext(nc) as tc:
        with tc.tile_pool(name="const", bufs=1) as cpool, \
             tc.tile_pool(name="feats", bufs=1) as fpool, \
             tc.tile_pool(name="big", bufs=1) as bigp, \
             tc.tile_pool(name="dram", bufs=2,
                          space=bass.MemorySpace.DRAM) as dpool, \
             tc.tile_pool(name="glue", bufs=2) as glue:
            # --- feats for fc1/sk (host-computed), 4 strip DMAs
            feats1_t = fpool.tile([128, (DIN // NQ) * B], BF16, tag="f1")
            for g in range(NQ):
                nc.sync.dma_start(feats1_t[32 * g:32 * g + 8, :],
                                  d["feats1"].ap()[8 * g:8 * g + 8, :])

            # --- sk emitter: first half runs before fc1 (covers the E1
            # expansion), second half after AG1 (covers the collective)
            ep_sk_cm = tc.tile_pool(name="w2e_sk", bufs=1)
            e_sk = ep_sk_cm.__enter__().tile([128, 16 * DIN * 4], BF16,
                                             tag="esk")
            sk_em = _KanEmitter(tc, feats1_t, d["w1sk"], e_sk, GEO_SK, "sk")
            sk_em.prefetch([0, 1])
            _emit_w2_expand(nc, e_sk, d["w2sk"], GEO_SK)

            # --- prefetch fc1's first two W1 blocks BEFORE the E1 fills so
            # stage-1 is never queued behind them on SP
            fc1_em = None  # created after prefetches to keep SP queue clean
            w1p_fc1_cm = tc.tile_pool(name="fc1_w1", bufs=2)
            w1p_fc1 = w1p_fc1_cm.__enter__()
            fc1_pre = {blk: _emit_w1_block(nc, w1p_fc1, d["w1fc1"], GEO_FC1,
                                           blk) for blk in range(2)}

            # --- expanded block-diagonal W2 for fc1
            ep_fc1_cm = tc.tile_pool(name="w2e_fc1", bufs=1)
            e_fc1 = ep_fc1_cm.__enter__().tile([128, 32 * DIN * 8], BF16,
                                               tag="efc1")
            _emit_w2_expand(nc, e_fc1, d["w2fc1"], GEO_FC1, split_memset=True)

            # --- constants (all consumed at layer tails — emit after the
            # start-critical DMA streams so they never delay them)
            bias = {}
            for nm, rows in (("b2fc1", 32), ("b2fc2", 16), ("b2sk", 16),
                             ("g1b", 128), ("g2b", 128), ("bnw", 128),
                             ("bnb", 128)):
                t = cpool.tile([rows, 1], F32, tag=nm)
                nc.sync.dma_start(t[:], d[nm].ap())
                bias[nm] = t
            g1w_t = cpool.tile([128, 128], F32, tag="g1w")
            nc.sync.dma_start(g1w_t[:], d["g1wT"].ap())
            g2w_t = cpool.tile([128, 128], F32, tag="g2w")
            nc.sync.dma_start(g2w_t[:], d["g2wT"].ap())


            # --- sk first half (covers the E1 build latency)
            sk_em.emit_blocks([0, 1], a_bufs=12)

            # --- fc1 (out cols 32c..32c+32), elu fused into output hook
            h1c = bigp.tile([32, 128], F32, tag="h1c")

            def elu_hook(out2, b2_t, out_sb):
                z = glue.tile([32, 128], F32, tag="eluz")
                nc.vector.tensor_scalar(z[:], out2[:], b2_t, None, op0=ALU.add)
                zn = glue.tile([32, 128], F32, tag="elun")
                nc.vector.tensor_scalar(zn[:], z[:], 0.0, None, op0=ALU.min)
                e = glue.tile([32, 128], F32, tag="elue")
                nc.scalar.activation(e[:], zn[:], AF.Exp)
                r = glue.tile([32, 128], F32, tag="elur")
                nc.vector.tensor_scalar(r[:], z[:], 0.0, None, op0=ALU.max)
                nc.vector.scalar_tensor_tensor(
                    out_sb, e[:], -1.0, r[:], op0=ALU.add, op1=ALU.add)

            fc1_em = _KanEmitter(tc, feats1_t, d["w1fc1"], e_fc1, GEO_FC1,
                                 "fc1", w1_pool=(w1p_fc1_cm, w1p_fc1))
            fc1_em.w1_pre = fc1_pre
            fc1_em.emit_blocks(range(fc1_em.nblk))
            fc1_em.finish(bias["b2fc1"][:], h1c[:], act_hook=elu_hook,
                          close_w1=False)
            ep_fc1_cm.__exit__(None, None, None)  # e_fc1 dead; reuse space
            w1p_fc1_cm.__exit__(None, None, None)

            # --- expand fc2's W2 (fills run under AG1 / sk second half),
            # and prefetch fc2's first W1 blocks in the same window
            ep_fc2_cm = tc.tile_pool(name="w2e_fc2", bufs=1)
            e_fc2 = ep_fc2_cm.__enter__().tile([128, 16 * DH * 4], BF16,
                                               tag="efc2")
            _emit_w2_expand(nc, e_fc2, d["w2fc2"], GEO_FC2)
            w1p_fc2_cm = tc.tile_pool(name="fc2_w1", bufs=2)
            w1p_fc2 = w1p_fc2_cm.__enter__()
            fc2_pre = {blk: _emit_w1_block(nc, w1p_fc2, d["w1fc2"], GEO_FC2,
                                           blk) for blk in range(2)}

            # --- AllGather #1: h1c [32,B] -> h1 [DH,B]
            ag1_in = dpool.tile([32, 128], F32, tag="ag1i")
            ag1_out = dpool.tile([DH, 128], F32, tag="ag1o")
            nc.sync.dma_start(ag1_in[:], h1c[:])
            nc.gpsimd.collective_compute(
                "AllGather", ALU.bypass, replica_groups=groups,
                ins=[ag1_in[:]], outs=[ag1_out[:]])

            # --- sk second half (covers the AllGather + trig chain)
            skc = bigp.tile([32, 128], F32, tag="catc")
            sk_tmp = bigp.tile([16, 128], F32, tag="sktmp")
            sk_em.emit_blocks([2, 3])
            sk_em.finish(bias["b2sk"][:], sk_tmp[:], close_w1=False)
            nc.sync.dma_start(skc[16:32, :], sk_tmp[:])

            # --- fc2 feature build (on-device trig with range reduction)
            # h1t: AG1 result as 2 tiles of [128(i), 128(b)] f32
            h1t = [bigp.tile([128, 128], F32, tag=f"h1t{hf}", name=f"h1t{hf}")
                   for hf in range(2)]
            for hf in range(2):
                nc.sync.dma_start(h1t[hf][:],
                                  ag1_out[128 * hf:128 * hf + 128, :])
            # feats2 strip g rows 32g+f = feature f of i in [64g, 64g+64)
            feats2_t = fpool.tile([128, (DH // NQ) * B], BF16, tag="f2")
            nc.gpsimd.memset(feats2_t[:], 1.0)  # const-1 row background
            # bounce DRAM tensor: row f = feature f, free (i in 256, b);
            # only 7 feature rows — the const-1 row stays from the memset
            f2d = dpool.tile([7, DH * B], BF16, tag="f2d")

            def store_feat_row(s, hf, src_bf16):
                # src [128(i), 128(b)] -> f2d[s, hf*16384 + i*128 + b]
                dst = f2d[s:s + 1, hf * 16384:(hf + 1) * 16384]
                vt = dst.rearrange("one (p b) -> (one p) b", p=128)
                nc.sync.dma_start(vt, src_bf16)

            for hf in range(2):
                c0 = glue.tile([128, 128], BF16, tag="c0")
                nc.vector.tensor_copy(c0[:], h1t[hf][:])
                store_feat_row(0, hf, c0[:])
                for s in range(NSIN):
                    sc = (2.0**s) / TWO_PI
                    y = glue.tile([128, 128], F32, tag="rry")
                    nc.vector.tensor_scalar(y[:], h1t[hf][:], sc, None,
                                            op0=ALU.mult)
                    kq = glue.tile([128, 128], F32, tag="rrk")
                    nc.vector.tensor_scalar(kq[:], y[:], MAGIC, -MAGIC,
                                            op0=ALU.add, op1=ALU.add)
                    fr = glue.tile([128, 128], F32, tag="rrf")
                    nc.vector.tensor_tensor(fr[:], y[:], kq[:],
                                            op=ALU.subtract)
                    sn = glue.tile([128, 128], BF16, tag="rrs")
                    nc.scalar.activation(sn[:], fr[:], AF.Sin, scale=TWO_PI)
                    store_feat_row(1 + s, hf, sn[:])
                    # cos(2pi y) = sin(2pi wrap(y + 1/4)): second range
                    # reduction keeps the Sin argument within +-pi
                    y2 = glue.tile([128, 128], F32, tag="rry2")
                    nc.vector.tensor_scalar(y2[:], y[:], 0.25, None,
                                            op0=ALU.add)
                    kq2 = glue.tile([128, 128], F32, tag="rrk2")
                    nc.vector.tensor_scalar(kq2[:], y2[:], MAGIC, -MAGIC,
                                            op0=ALU.add, op1=ALU.add)
                    frc = glue.tile([128, 128], F32, tag="rrfc")
                    nc.vector.tensor_tensor(frc[:], y2[:], kq2[:],
                                            op=ALU.subtract)
                    cs = glue.tile([128, 128], BF16, tag="rrc")
                    nc.scalar.activation(cs[:], frc[:], AF.Sin, scale=TWO_PI)
                    store_feat_row(4 + s, hf, cs[:])
            # read back: strip g <- features of i in [64g, 64g+64)
            for g in range(NQ):
                nc.sync.dma_start(
                    feats2_t[32 * g:32 * g + 7, :],
                    f2d[0:7, g * 64 * 128:(g + 1) * 64 * 128])

            # --- fc2 (out cols 16c..16c+16)
            _emit_kan_layer(tc, feats2_t, d["w1fc2"], e_fc2,
                            bias["b2fc2"][:], skc[0:16, :], GEO_FC2, "fc2",
                            w1_pool=(w1p_fc2_cm, w1p_fc2), w1_pre=fc2_pre)
            ep_fc2_cm.__exit__(None, None, None)
            sk_em.w1p_cm.__exit__(None, None, None)
            ep_sk_cm.__exit__(None, None, None)

            # --- AllGather #2: [fc2c; skc] [32,B] -> [2*DOUT, B]
            ag2_in = dpool.tile([32, 128], F32, tag="ag2i")
            ag2_out = dpool.tile([2 * DOUT, 128], F32, tag="ag2o")
            nc.sync.dma_start(ag2_in[:], skc[:])
            nc.gpsimd.collective_compute(
                "AllGather", ALU.bypass, replica_groups=groups,
                ins=[ag2_in[:]], outs=[ag2_out[:]])
            h2t = bigp.tile([128, 128], F32, tag="h2t")
            rest = bigp.tile([128, 128], F32, tag="rest")
            # h2t first: the GLU matmuls gate on it; rest is consumed later
            for c in range(NCORES):
                nc.sync.dma_start(h2t[16 * c:16 * c + 16, :],
                                  ag2_out[32 * c:32 * c + 16, :])
            for c in range(NCORES):
                nc.sync.dma_start(rest[16 * c:16 * c + 16, :],
                                  ag2_out[32 * c + 16:32 * c + 32, :])

            # --- GLU gate (replicated, fp32 matmuls)
            gp_cm = tc.tile_pool(name="gpsum", bufs=1,
                                 space=bass.MemorySpace.PSUM)
            gpsum = gp_cm.__enter__()
            z1 = gpsum.tile([128, 128], F32, tag="z1")
            nc.tensor.matmul(z1[:], g1w_t[:], h2t[:], start=True, stop=True)
            z2 = gpsum.tile([128, 128], F32, tag="z2")
            nc.tensor.matmul(z2[:], g2w_t[:], h2t[:], start=True, stop=True)
            sig = glue.tile([128, 128], F32, tag="sig")
            nc.scalar.activation(sig[:], z1[:], AF.Sigmoid, bias=bias["g1b"][:])
            z2b = glue.tile([128, 128], F32, tag="z2b")
            nc.vector.tensor_scalar(z2b[:], z2[:], bias["g2b"][:], None,
                                    op0=ALU.add)
            h3 = glue.tile([128, 128], F32, tag="h3")
            nc.vector.tensor_tensor(h3[:], sig[:], z2b[:], op=ALU.mult)
            h4 = glue.tile([128, 128], F32, tag="h4")
            nc.vector.tensor_tensor(h4[:], h3[:], rest[:], op=ALU.add)

            # --- BatchNorm over batch (free dim) — replicated
            mean = glue.tile([128, 1], F32, tag="mean")
            nc.vector.tensor_reduce(mean[:], h4[:], axis=mybir.AxisListType.X,
                                    op=ALU.add)
            nc.vector.tensor_scalar(mean[:], mean[:], 1.0 / B, None,
                                    op0=ALU.mult)
            sq = glue.tile([128, 128], F32, tag="sq")
            ssq = glue.tile([128, 1], F32, tag="ssq")
            nc.scalar.activation(sq[:], h4[:], AF.Square, accum_out=ssq[:])
            # var = ssq/B - mean^2 + eps folded into two DVE ops
            var = glue.tile([128, 1], F32, tag="var")
            nc.vector.tensor_scalar(var[:], ssq[:], 1.0 / B, None,
                                    op0=ALU.mult)
            m2 = glue.tile([128, 1], F32, tag="m2")
            nc.vector.tensor_tensor(m2[:], mean[:], mean[:], op=ALU.mult)
            nc.vector.tensor_tensor(var[:], var[:], m2[:], op=ALU.subtract)
            nc.vector.tensor_scalar(var[:], var[:], BN_EPS, None, op0=ALU.add)
            rec = glue.tile([128, 1], F32, tag="rec")
            nc.vector.reciprocal(rec[:], var[:])
            rstd = glue.tile([128, 1], F32, tag="rstd")
            nc.scalar.activation(rstd[:], rec[:], AF.Sqrt)
            scl = glue.tile([128, 1], F32, tag="scl")
            nc.vector.tensor_tensor(scl[:], rstd[:], bias["bnw"][:],
                                    op=ALU.mult)
            outt = glue.tile([128, 128], F32, tag="outt")
            nc.vector.tensor_scalar(outt[:], h4[:], mean[:], scl[:],
                                    op0=ALU.subtract, op1=ALU.mult)
            nc.vector.tensor_scalar(outt[:], outt[:], bias["bnb"][:], None,
                                    op0=ALU.add)
            nc.sync.dma_start(out_d.ap(), outt[:])
            gp_cm.__exit__(None, None, None)
    nc.compile()
    return nc


_CACHED_NC = None


def kernel(**inputs):
    global _CACHED_NC
    in_maps = prep_in_maps(inputs)
    if _CACHED_NC is None:
        _CACHED_NC = build_program()
    res = bass_utils.run_bass_kernel_spmd(_CACHED_NC, in_maps,
                                          core_ids=list(range(NCORES)))
    out_t = np.asarray(res.results[0]["outT"], np.float32)
    return np.ascontiguousarray(out_t.T)


if __name__ == "__main__":
    rng = np.random.default_rng(0)
    fake = {"x": rng.normal(size=(B, DIN)).astype(np.float32)}
    print("module loads OK")
